# revision 23
# baseline (speedup 1.0000x reference)
"""Blockdiag butterfly (Monarch) linear on 8 TRN2 NeuronCores.

Math (see reference): x:[B,4096] f32, w1:[4,192,1024], w2:[4,1024,192], bias:[4096]
  stage1: out1[b,k,q] = sum_p x[b, k*1024+p] * w1[k,q,p]          (q = l*48+j)
  block transpose: out1t[b,l,r] = out1[b,k,l*48+j], r = k*48+j
  stage2: out[b, l*1024+s] = sum_r out1t[b,l,r] * w2[l,s,r] + bias

Sharding: pure data-parallel over the batch dim (2048 tokens/core),
weights replicated.  x is transposed and cast to bf16 host-side, so the
device never transposes and every matmul runs at the 1-cycle/row bf16
rate (fp32 is 4 cycles/row on TRN2) while halving HBM traffic.  Both
stages contract over the partition dim with no on-chip transposes:
stage 1 is weight-stationary producing psum[q, b], which is exactly the
[r, b] orientation stage 2 needs as its stationary operand.

The per-block width 48 is zero-padded to 64 host-side, so the
intermediate lives in clean 64-row partition groups and the
stage1->stage2 block transpose becomes aligned psum->SBUF copies
(cast to bf16 on the way).  Bias is planted in w2's last padding row
(255) against a constant-1.0 row memset into the intermediate, making
psum evacuation a plain cast-copy split across DVE/ACT.  Output is
written bf16 and expanded to f32 host-side (rel tolerance is 2e-2;
bf16 end-to-end error is ~1e-3 of output scale).
"""

import numpy as np

NB1, NB2, B1 = 4, 4, 48
B1P = 64
IN_F, OUT_F = 4096, 4096
BATCH = 16384
N_CORES = 8
B_LOCAL = BATCH // N_CORES
P = 128
NQP = NB2 * B1P              # 256
NRP = NB1 * B1P              # 256
PC = IN_F // NB1 // P        # 8
BT = 512
NBT = B_LOCAL // BT          # 4
S = OUT_F // NB2             # 1024

_CACHE = {}


def _np_bf16():
    import ml_dtypes
    return ml_dtypes.bfloat16


def _emit(nc, xt, w1t, w2t, bias, out, reps=1, x_resident=False,
          skip_out=False, nmov=None, hw_loop=False, mm_only=False):
    import concourse.mybir as mybir
    import concourse.tile as tile

    f32 = mybir.dt.float32
    bf16 = mybir.dt.bfloat16

    w1_v = w1t.rearrange("k (pc pi) q -> pi k pc q", pc=PC, pi=P)
    w2_v = w2t.rearrange("l (rc ri) s -> ri l rc s", rc=NRP // P, ri=P)

    with tile.TileContext(nc) as tc:
        with (
            tc.tile_pool(name="consts", bufs=1) as consts,
            tc.tile_pool(name="xin", bufs=2) as xin,
            tc.tile_pool(name="mid", bufs=2) as mid,
            tc.tile_pool(name="outp", bufs=2) as outp,
            tc.tile_pool(name="ps1", bufs=2, space="PSUM") as ps1,
            tc.tile_pool(name="ps2", bufs=2, space="PSUM") as ps2,
        ):
            # weights go out on the ACT/DVE HWDGE rings so the SP ring is
            # free for the first x-tile load
            w1_sb = consts.tile([P, NB1, PC, NQP], bf16)
            # per-k chunks: the first stage-1 matmul only needs w1[k=0]
            for k in range(NB1):
                nc.scalar.dma_start(w1_sb[:, k, :, :], w1_v[:, k, :, :])
            w2_sb = consts.tile([P, NB2, NRP // P, S], bf16)
            nc.scalar.dma_start(w2_sb[:], w2_v)
            # keep the bias input alive so the NEFF keeps the tensor
            bias_sb = consts.tile([1, OUT_F], f32)
            nc.scalar.dma_start(bias_sb[:], bias[None, :])

            xres = None
            if x_resident:
                # perf-bisection mode: one x tile parked in SBUF, reused for
                # every block-tile so the rep loop has no input DMA traffic
                # (timing-only -- output values are wrong for 3 of 4 tiles)
                xres = consts.tile([P, NB1, PC, BT], bf16, name="xr")
                nc.sync.dma_start(xres[:], xt[0])

            def one_tile(bt):
                bsl = slice(bt * BT, (bt + 1) * BT)
                # ---- stage 1 ----
                if x_resident:
                    xk = xres
                else:
                    xk = xin.tile([P, NB1, PC, BT], bf16, tag="xk", name="xk")
                    # one DMA per k-block (1 MB each) so the first stage-1
                    # matmul group only waits for a quarter of the tile's x;
                    # xt is pre-laid-out so each partition's slice is one
                    # contiguous 8 KB descriptor
                    for k in range(NB1):
                        nc.sync.dma_start(xk[:, k, :, :], xt[bt, :, k, :, :])
                o1 = [
                    mid.tile([P, NB2, BT], bf16, tag="o1a", name="o1a"),
                    mid.tile([P, NB2, BT], bf16, tag="o1b", name="o1b"),
                ]
                if nmov or mm_only:
                    # timing-bisect modes leave parts of o1 unwritten; zero
                    # it so stage 2 never reads uninitialized SBUF
                    nc.gpsimd.memset(o1[0][:, :, :], 0.0)
                    nc.gpsimd.memset(o1[1][:, :, :], 0.0)
                # constant-1.0 rows facing w2's bias row: gpsimd needs a
                # 32-aligned start partition, so set 96:128 and let the k=3
                # copy overwrite 96:112 with real data; rows 112:126 face
                # w2 zero-padding rows and 127 faces the bias row
                nc.gpsimd.memset(o1[1][96:2 * B1P, :, :], 1.0)
                nm = nmov or BT
                for k in range(NB1):
                    pq = ps1.tile([P, 2, BT], f32, tag="pq", name="pq")
                    for qc in range(2):
                        for pc in range(PC):
                            nc.tensor.matmul(
                                pq[:, qc, 0:nm],
                                w1_sb[:, k, pc, qc * P:(qc + 1) * P],
                                xk[:, k, pc, 0:nm],
                                start=(pc == 0), stop=(pc == PC - 1),
                            )
                    half = k // 2
                    r0 = (k % 2) * B1P
                    nrow = B1P if not (half == 1 and k % 2 == 1) else B1
                    for l in ([] if mm_only else list(range(NB2))):
                        dst = o1[half][r0:r0 + nrow, l, 0:nm]
                        src = pq[(l % 2) * B1P:(l % 2) * B1P + nrow, l // 2, 0:nm]
                        if l % 2 == 0:
                            nc.vector.tensor_copy(out=dst, in_=src)
                        else:
                            nc.scalar.copy(dst, src)
                # ---- stage 2 ----
                nm2 = nmov or 512
                for bi in range(BT // P):
                    b0 = bt * BT + bi * P
                    bloc = slice(bi * P, (bi + 1) * P)
                    ob = outp.tile([P, NB2, S], bf16, tag="ob", name="ob")
                    for l in range(NB2):
                        # one 2-bank psum group per l: both s-halves
                        # accumulate both r-halves before a single
                        # evacuation, halving psum group boundaries
                        ps = ps2.tile([P, 2, 512], f32, tag="ps2", name="ps2")
                        for half in range(2):
                            for sh in range(S // 512):
                                ns2 = slice(sh * 512, sh * 512 + nm2)
                                nc.tensor.matmul(
                                    ps[:, sh, 0:nm2], o1[half][:, l, bloc],
                                    w2_sb[:, l, half, ns2],
                                    start=(half == 0), stop=(half == 1),
                                )
                        for sh in ([] if mm_only else list(range(S // 512))):
                            ns2 = slice(sh * 512, sh * 512 + nm2)
                            if (l + sh) % 2 == 0:
                                nc.vector.tensor_copy(
                                    out=ob[:, l, ns2], in_=ps[:, sh, 0:nm2])
                            else:
                                nc.scalar.copy(ob[:, l, ns2], ps[:, sh, 0:nm2])
                    if not skip_out:
                        # output stores alternate between the Pool (SWDGE)
                        # and SP rings so neither queue serializes them
                        if bi % 2 == 0:
                            nc.gpsimd.dma_start(out[b0:b0 + P, :], ob[:])
                        else:
                            nc.sync.dma_start(out[b0:b0 + P, :], ob[:])
                return ob, o1

            if hw_loop and reps > 1:
                # hardware loop over reps: constant NEFF size, instructions
                # fetched once -- the For_i reset barrier drains the
                # pipeline between reps
                with tc.For_i(0, reps):
                    for bt in range(NBT):
                        ob, o1 = one_tile(bt)
            else:
                for it in range(NBT * reps):
                    ob, o1 = one_tile(it % NBT)
            if skip_out:
                # keep the output tensor written so the NEFF retains it
                if mm_only:
                    nc.sync.dma_start(out[0:P, 0:512], o1[0][:, 0, 0:512])
                else:
                    nw = nmov or 512
                    nc.sync.dma_start(out[0:P, 0:nw], ob[:, 0, 0:nw])


def _build(reps=1, x_resident=False, skip_out=False, nmov=None,
           hw_loop=False, mm_only=False):
    import concourse.bacc as bacc
    import concourse.mybir as mybir

    # Bacc (not plain Bass): its compile() legalizes semaphore waits
    # (move_matmul_waits_to_ldweights + generate_event_semaphores) --
    # walrus allows at most one sync wait per instruction.
    suffix = ("_xr" if x_resident else "") + ("_no" if skip_out else "")
    if nmov:
        suffix += f"_n{nmov}"
    if hw_loop:
        suffix += "_hl"
    if mm_only:
        suffix += "_mm"
    nc = bacc.Bacc(name=f"bfly_r{reps}{suffix}")
    bf16 = mybir.dt.bfloat16
    xt = nc.dram_tensor("xt", [NBT, P, NB1, PC, BT], bf16,
                        kind="ExternalInput")
    w1t = nc.dram_tensor("w1t", [NB1, IN_F // NB1, NQP], bf16, kind="ExternalInput")
    w2t = nc.dram_tensor("w2t", [NB2, NRP, S], bf16, kind="ExternalInput")
    bias = nc.dram_tensor("bias", [OUT_F], mybir.dt.float32, kind="ExternalInput")
    out = nc.dram_tensor("out", [B_LOCAL, OUT_F], bf16, kind="ExternalOutput")
    _emit(nc, xt[:], w1t[:], w2t[:], bias[:], out[:], reps=reps,
          x_resident=x_resident, skip_out=skip_out, nmov=nmov,
          hw_loop=hw_loop, mm_only=mm_only)
    nc.compile()
    return nc


def get_nc(reps=1, x_resident=False, skip_out=False, nmov=None,
           hw_loop=False, mm_only=False):
    key = ("nc", reps, x_resident, skip_out, nmov, hw_loop, mm_only)
    if key not in _CACHE:
        _CACHE[key] = _build(reps, x_resident, skip_out, nmov, hw_loop,
                             mm_only)
    return _CACHE[key]


def prep_weights(w1_bfly, w2_bfly, bias):
    """Pad the per-block width 48 -> 64, transpose for the device layout,
    cast to bf16, and plant bias in w2t's last padding row."""
    bf16 = _np_bf16()
    w1t = np.zeros((NB1, IN_F // NB1, NQP), dtype=np.float32)
    w1t_v = w1t.reshape(NB1, IN_F // NB1, NB2, B1P)
    w1t_v[:, :, :, :B1] = (
        np.asarray(w1_bfly, np.float32)
        .transpose(0, 2, 1).reshape(NB1, IN_F // NB1, NB2, B1)
    )
    w2t = np.zeros((NB2, NRP, S), dtype=np.float32)
    w2t_v = w2t.reshape(NB2, NB1, B1P, S)
    w2t_v[:, :, :B1, :] = (
        np.asarray(w2_bfly, np.float32)
        .transpose(0, 2, 1).reshape(NB2, NB1, B1, S)
    )
    w2t[:, NRP - 1, :] = np.asarray(bias, np.float32).reshape(NB2, S)
    return w1t.astype(bf16), w2t.astype(bf16)


def _prep_inputs(x, w1_bfly, w2_bfly, bias):
    bf16 = _np_bf16()
    bias = np.ascontiguousarray(np.asarray(bias, np.float32))
    w1t, w2t = prep_weights(w1_bfly, w2_bfly, bias)
    xb = np.asarray(x, np.float32).astype(bf16)
    in_maps = []
    for c in range(N_CORES):
        xc = xb[c * B_LOCAL:(c + 1) * B_LOCAL]
        # [bt, pi, k, pc, b]: each partition's per-tile slice lands as one
        # contiguous run in DRAM, so x DMAs use 8 KB descriptors
        xs = np.ascontiguousarray(
            xc.reshape(NBT, BT, NB1, PC, P).transpose(0, 4, 2, 3, 1))
        in_maps.append({"xt": xs, "w1t": w1t, "w2t": w2t, "bias": bias})
    return in_maps


def kernel(x, w1_bfly, w2_bfly, bias):
    from concourse.bass_utils import run_bass_kernel_spmd

    nc = get_nc()
    in_maps = _prep_inputs(np.asarray(x), np.asarray(w1_bfly),
                           np.asarray(w2_bfly), np.asarray(bias))
    res = run_bass_kernel_spmd(nc, in_maps, list(range(N_CORES)), trace=False)
    return np.concatenate(
        [np.asarray(res.results[c]["out"], np.float32) for c in range(N_CORES)],
        axis=0)


# revision 24
# speedup vs baseline: 1.0338x; 1.0338x over previous
"""Blockdiag butterfly (Monarch) linear on 8 TRN2 NeuronCores.

Math (see reference): x:[B,4096] f32, w1:[4,192,1024], w2:[4,1024,192], bias:[4096]
  stage1: out1[b,k,q] = sum_p x[b, k*1024+p] * w1[k,q,p]          (q = l*48+j)
  block transpose: out1t[b,l,r] = out1[b,k,l*48+j], r = k*48+j
  stage2: out[b, l*1024+s] = sum_r out1t[b,l,r] * w2[l,s,r] + bias

Sharding: pure data-parallel over the batch dim (2048 tokens/core),
weights replicated.  x is transposed and cast to bf16 host-side, so the
device never transposes and every matmul runs at the 1-cycle/row bf16
rate (fp32 is 4 cycles/row on TRN2) while halving HBM traffic.  Both
stages contract over the partition dim with no on-chip transposes:
stage 1 is weight-stationary producing psum[q, b], which is exactly the
[r, b] orientation stage 2 needs as its stationary operand.

The per-block width 48 is zero-padded to 64 host-side, so the
intermediate lives in clean 64-row partition groups and the
stage1->stage2 block transpose becomes aligned psum->SBUF copies
(cast to bf16 on the way).  Bias is planted in w2's last padding row
(255) against a constant-1.0 row memset into the intermediate, making
psum evacuation a plain cast-copy split across DVE/ACT.  Output is
written bf16 and expanded to f32 host-side (rel tolerance is 2e-2;
bf16 end-to-end error is ~1e-3 of output scale).
"""

import numpy as np

NB1, NB2, B1 = 4, 4, 48
B1P = 64
IN_F, OUT_F = 4096, 4096
BATCH = 16384
N_CORES = 8
B_LOCAL = BATCH // N_CORES
P = 128
NQP = NB2 * B1P              # 256
NRP = NB1 * B1P              # 256
PC = IN_F // NB1 // P        # 8
BT = 512
NBT = B_LOCAL // BT          # 4
S = OUT_F // NB2             # 1024

_CACHE = {}


def _np_bf16():
    import ml_dtypes
    return ml_dtypes.bfloat16


def _emit(nc, xt, w1t, w2t, bias, out, reps=1, x_resident=False,
          skip_out=False, nmov=None, hw_loop=False, mm_only=False):
    import concourse.mybir as mybir
    import concourse.tile as tile

    f32 = mybir.dt.float32
    bf16 = mybir.dt.bfloat16

    w1_v = w1t.rearrange("k (pc pi) q -> pi k pc q", pc=PC, pi=P)
    w2_v = w2t.rearrange("l (rc ri) s -> ri l rc s", rc=NRP // P, ri=P)

    with tile.TileContext(nc) as tc:
        with (
            tc.tile_pool(name="consts", bufs=1) as consts,
            tc.tile_pool(name="xin", bufs=2) as xin,
            tc.tile_pool(name="mid", bufs=2) as mid,
            tc.tile_pool(name="outp", bufs=2) as outp,
            tc.tile_pool(name="ps1", bufs=2, space="PSUM") as ps1,
            tc.tile_pool(name="ps2", bufs=2, space="PSUM") as ps2,
        ):
            # weights go out on the ACT/DVE HWDGE rings so the SP ring is
            # free for the first x-tile load
            w1_sb = consts.tile([P, NB1, PC, NQP], bf16)
            # per-k chunks: the first stage-1 matmul only needs w1[k=0]
            for k in range(NB1):
                nc.scalar.dma_start(w1_sb[:, k, :, :], w1_v[:, k, :, :])
            w2_sb = consts.tile([P, NB2, NRP // P, S], bf16)
            nc.scalar.dma_start(w2_sb[:], w2_v)
            # keep the bias input alive so the NEFF keeps the tensor
            bias_sb = consts.tile([1, OUT_F], f32)
            nc.scalar.dma_start(bias_sb[:], bias[None, :])

            xres = None
            if x_resident:
                # perf-bisection mode: one x tile parked in SBUF, reused for
                # every block-tile so the rep loop has no input DMA traffic
                # (timing-only -- output values are wrong for 3 of 4 tiles)
                xres = consts.tile([P, NB1, PC, BT], bf16, name="xr")
                nc.sync.dma_start(xres[:], xt[0])

            def one_tile(bt):
                bsl = slice(bt * BT, (bt + 1) * BT)
                # ---- stage 1 ----
                if x_resident:
                    xk = xres
                else:
                    xk = xin.tile([P, NB1, PC, BT], bf16, tag="xk", name="xk")
                    # one DMA per k-block (1 MB each) so the first stage-1
                    # matmul group only waits for a quarter of the tile's x;
                    # xt is pre-laid-out so each partition's slice is one
                    # contiguous 8 KB descriptor
                    for k in range(NB1):
                        nc.sync.dma_start(xk[:, k, :, :], xt[bt, :, k, :, :])
                o1 = [
                    mid.tile([P, NB2, BT], bf16, tag="o1a", name="o1a"),
                    mid.tile([P, NB2, BT], bf16, tag="o1b", name="o1b"),
                ]
                if nmov or mm_only:
                    # timing-bisect modes leave parts of o1 unwritten; zero
                    # it so stage 2 never reads uninitialized SBUF
                    nc.gpsimd.memset(o1[0][:, :, :], 0.0)
                    nc.gpsimd.memset(o1[1][:, :, :], 0.0)
                # constant-1.0 rows facing w2's bias row: gpsimd needs a
                # 32-aligned start partition, so set 96:128 and let the k=3
                # copy overwrite 96:112 with real data; rows 112:126 face
                # w2 zero-padding rows and 127 faces the bias row
                nc.gpsimd.memset(o1[1][96:2 * B1P, :, :], 1.0)
                nm = nmov or BT
                for k in range(NB1):
                    pq = ps1.tile([P, 2, BT], f32, tag="pq", name="pq")
                    for qc in range(2):
                        for pc in range(PC):
                            nc.tensor.matmul(
                                pq[:, qc, 0:nm],
                                w1_sb[:, k, pc, qc * P:(qc + 1) * P],
                                xk[:, k, pc, 0:nm],
                                start=(pc == 0), stop=(pc == PC - 1),
                            )
                    half = k // 2
                    r0 = (k % 2) * B1P
                    nrow = B1P if not (half == 1 and k % 2 == 1) else B1
                    for l in ([] if mm_only else list(range(NB2))):
                        dst = o1[half][r0:r0 + nrow, l, 0:nm]
                        src = pq[(l % 2) * B1P:(l % 2) * B1P + nrow, l // 2, 0:nm]
                        if l % 2 == 0:
                            nc.vector.tensor_copy(out=dst, in_=src)
                        else:
                            nc.scalar.copy(dst, src)
                # ---- stage 2 ----
                nm2 = nmov or 512
                for bi in range(BT // P):
                    b0 = bt * BT + bi * P
                    bloc = slice(bi * P, (bi + 1) * P)
                    ob = outp.tile([P, NB2, S], bf16, tag="ob", name="ob")
                    for l in range(NB2):
                        # one 2-bank psum group per l: both s-halves
                        # accumulate both r-halves before a single
                        # evacuation, halving psum group boundaries
                        ps = ps2.tile([P, 2, 512], f32, tag="ps2", name="ps2")
                        for half in range(2):
                            for sh in range(S // 512):
                                ns2 = slice(sh * 512, sh * 512 + nm2)
                                nc.tensor.matmul(
                                    ps[:, sh, 0:nm2], o1[half][:, l, bloc],
                                    w2_sb[:, l, half, ns2],
                                    start=(half == 0), stop=(half == 1),
                                )
                        for sh in ([] if mm_only else list(range(S // 512))):
                            ns2 = slice(sh * 512, sh * 512 + nm2)
                            if (l + sh) % 2 == 0:
                                nc.vector.tensor_copy(
                                    out=ob[:, l, ns2], in_=ps[:, sh, 0:nm2])
                            else:
                                nc.scalar.copy(ob[:, l, ns2], ps[:, sh, 0:nm2])
                    if not skip_out:
                        # all output stores on the Pool (SWDGE) ring: the SP
                        # ring is FIFO, so stores there would delay the next
                        # tile's x loads behind them
                        nc.gpsimd.dma_start(out[b0:b0 + P, :], ob[:])
                return ob, o1

            if hw_loop and reps > 1:
                # hardware loop over reps: constant NEFF size, instructions
                # fetched once -- the For_i reset barrier drains the
                # pipeline between reps
                with tc.For_i(0, reps):
                    for bt in range(NBT):
                        ob, o1 = one_tile(bt)
            else:
                for it in range(NBT * reps):
                    ob, o1 = one_tile(it % NBT)
            if skip_out:
                # keep the output tensor written so the NEFF retains it
                if mm_only:
                    nc.sync.dma_start(out[0:P, 0:512], o1[0][:, 0, 0:512])
                else:
                    nw = nmov or 512
                    nc.sync.dma_start(out[0:P, 0:nw], ob[:, 0, 0:nw])


def _build(reps=1, x_resident=False, skip_out=False, nmov=None,
           hw_loop=False, mm_only=False):
    import concourse.bacc as bacc
    import concourse.mybir as mybir

    # Bacc (not plain Bass): its compile() legalizes semaphore waits
    # (move_matmul_waits_to_ldweights + generate_event_semaphores) --
    # walrus allows at most one sync wait per instruction.
    suffix = ("_xr" if x_resident else "") + ("_no" if skip_out else "")
    if nmov:
        suffix += f"_n{nmov}"
    if hw_loop:
        suffix += "_hl"
    if mm_only:
        suffix += "_mm"
    nc = bacc.Bacc(name=f"bfly_r{reps}{suffix}")
    bf16 = mybir.dt.bfloat16
    xt = nc.dram_tensor("xt", [NBT, P, NB1, PC, BT], bf16,
                        kind="ExternalInput")
    w1t = nc.dram_tensor("w1t", [NB1, IN_F // NB1, NQP], bf16, kind="ExternalInput")
    w2t = nc.dram_tensor("w2t", [NB2, NRP, S], bf16, kind="ExternalInput")
    bias = nc.dram_tensor("bias", [OUT_F], mybir.dt.float32, kind="ExternalInput")
    out = nc.dram_tensor("out", [B_LOCAL, OUT_F], bf16, kind="ExternalOutput")
    _emit(nc, xt[:], w1t[:], w2t[:], bias[:], out[:], reps=reps,
          x_resident=x_resident, skip_out=skip_out, nmov=nmov,
          hw_loop=hw_loop, mm_only=mm_only)
    nc.compile()
    return nc


def get_nc(reps=1, x_resident=False, skip_out=False, nmov=None,
           hw_loop=False, mm_only=False):
    key = ("nc", reps, x_resident, skip_out, nmov, hw_loop, mm_only)
    if key not in _CACHE:
        _CACHE[key] = _build(reps, x_resident, skip_out, nmov, hw_loop,
                             mm_only)
    return _CACHE[key]


def prep_weights(w1_bfly, w2_bfly, bias):
    """Pad the per-block width 48 -> 64, transpose for the device layout,
    cast to bf16, and plant bias in w2t's last padding row."""
    bf16 = _np_bf16()
    w1t = np.zeros((NB1, IN_F // NB1, NQP), dtype=np.float32)
    w1t_v = w1t.reshape(NB1, IN_F // NB1, NB2, B1P)
    w1t_v[:, :, :, :B1] = (
        np.asarray(w1_bfly, np.float32)
        .transpose(0, 2, 1).reshape(NB1, IN_F // NB1, NB2, B1)
    )
    w2t = np.zeros((NB2, NRP, S), dtype=np.float32)
    w2t_v = w2t.reshape(NB2, NB1, B1P, S)
    w2t_v[:, :, :B1, :] = (
        np.asarray(w2_bfly, np.float32)
        .transpose(0, 2, 1).reshape(NB2, NB1, B1, S)
    )
    w2t[:, NRP - 1, :] = np.asarray(bias, np.float32).reshape(NB2, S)
    return w1t.astype(bf16), w2t.astype(bf16)


def _prep_inputs(x, w1_bfly, w2_bfly, bias):
    bf16 = _np_bf16()
    bias = np.ascontiguousarray(np.asarray(bias, np.float32))
    w1t, w2t = prep_weights(w1_bfly, w2_bfly, bias)
    xb = np.asarray(x, np.float32).astype(bf16)
    in_maps = []
    for c in range(N_CORES):
        xc = xb[c * B_LOCAL:(c + 1) * B_LOCAL]
        # [bt, pi, k, pc, b]: each partition's per-tile slice lands as one
        # contiguous run in DRAM, so x DMAs use 8 KB descriptors
        xs = np.ascontiguousarray(
            xc.reshape(NBT, BT, NB1, PC, P).transpose(0, 4, 2, 3, 1))
        in_maps.append({"xt": xs, "w1t": w1t, "w2t": w2t, "bias": bias})
    return in_maps


def kernel(x, w1_bfly, w2_bfly, bias):
    from concourse.bass_utils import run_bass_kernel_spmd

    nc = get_nc()
    in_maps = _prep_inputs(np.asarray(x), np.asarray(w1_bfly),
                           np.asarray(w2_bfly), np.asarray(bias))
    res = run_bass_kernel_spmd(nc, in_maps, list(range(N_CORES)), trace=False)
    return np.concatenate(
        [np.asarray(res.results[c]["out"], np.float32) for c in range(N_CORES)],
        axis=0)


# revision 25
# speedup vs baseline: 1.0605x; 1.0258x over previous
"""Blockdiag butterfly (Monarch) linear on 8 TRN2 NeuronCores.

Math (see reference): x:[B,4096] f32, w1:[4,192,1024], w2:[4,1024,192], bias:[4096]
  stage1: out1[b,k,q] = sum_p x[b, k*1024+p] * w1[k,q,p]          (q = l*48+j)
  block transpose: out1t[b,l,r] = out1[b,k,l*48+j], r = k*48+j
  stage2: out[b, l*1024+s] = sum_r out1t[b,l,r] * w2[l,s,r] + bias

Sharding: pure data-parallel over the batch dim (2048 tokens/core),
weights replicated.  x is transposed and cast to bf16 host-side, so the
device never transposes and every matmul runs at the 1-cycle/row bf16
rate (fp32 is 4 cycles/row on TRN2) while halving HBM traffic.  Both
stages contract over the partition dim with no on-chip transposes:
stage 1 is weight-stationary producing psum[q, b], which is exactly the
[r, b] orientation stage 2 needs as its stationary operand.

The per-block width 48 is zero-padded to 64 host-side, so the
intermediate lives in clean 64-row partition groups and the
stage1->stage2 block transpose becomes aligned psum->SBUF copies
(cast to bf16 on the way).  Bias is planted in w2's last padding row
(255) against a constant-1.0 row memset into the intermediate, making
psum evacuation a plain cast-copy split across DVE/ACT.  Output is
written bf16 and expanded to f32 host-side (rel tolerance is 2e-2;
bf16 end-to-end error is ~1e-3 of output scale).
"""

import numpy as np

NB1, NB2, B1 = 4, 4, 48
B1P = 64
IN_F, OUT_F = 4096, 4096
BATCH = 16384
N_CORES = 8
B_LOCAL = BATCH // N_CORES
P = 128
NQP = NB2 * B1P              # 256
NRP = NB1 * B1P              # 256
PC = IN_F // NB1 // P        # 8
BT = 512
NBT = B_LOCAL // BT          # 4
S = OUT_F // NB2             # 1024

_CACHE = {}


def _np_bf16():
    import ml_dtypes
    return ml_dtypes.bfloat16


def _emit(nc, xt, w1t, w2t, bias, out, reps=1, x_resident=False,
          skip_out=False, nmov=None, hw_loop=False, mm_only=False):
    import concourse.mybir as mybir
    import concourse.tile as tile

    f32 = mybir.dt.float32
    bf16 = mybir.dt.bfloat16

    w1_v = w1t.rearrange("k (pc pi) q -> pi k pc q", pc=PC, pi=P)
    w2_v = w2t.rearrange("l (rc ri) s -> ri l rc s", rc=NRP // P, ri=P)

    with tile.TileContext(nc) as tc:
        with (
            tc.tile_pool(name="consts", bufs=1) as consts,
            tc.tile_pool(name="xin", bufs=3) as xin,
            tc.tile_pool(name="mid", bufs=2) as mid,
            tc.tile_pool(name="outp", bufs=3) as outp,
            tc.tile_pool(name="ps1", bufs=2, space="PSUM") as ps1,
            tc.tile_pool(name="ps2", bufs=2, space="PSUM") as ps2,
        ):
            # weights go out on the ACT/DVE HWDGE rings so the SP ring is
            # free for the first x-tile load
            w1_sb = consts.tile([P, NB1, PC, NQP], bf16)
            # per-k chunks: the first stage-1 matmul only needs w1[k=0]
            for k in range(NB1):
                nc.scalar.dma_start(w1_sb[:, k, :, :], w1_v[:, k, :, :])
            w2_sb = consts.tile([P, NB2, NRP // P, S], bf16)
            nc.scalar.dma_start(w2_sb[:], w2_v)
            # keep the bias input alive so the NEFF keeps the tensor
            bias_sb = consts.tile([1, OUT_F], f32)
            nc.scalar.dma_start(bias_sb[:], bias[None, :])

            xres = None
            if x_resident:
                # perf-bisection mode: one x tile parked in SBUF, reused for
                # every block-tile so the rep loop has no input DMA traffic
                # (timing-only -- output values are wrong for 3 of 4 tiles)
                xres = consts.tile([P, NB1, PC, BT], bf16, name="xr")
                nc.sync.dma_start(xres[:], xt[0])

            def one_tile(bt):
                bsl = slice(bt * BT, (bt + 1) * BT)
                # ---- stage 1 ----
                if x_resident:
                    xk = xres
                else:
                    xk = xin.tile([P, NB1, PC, BT], bf16, tag="xk", name="xk")
                    # one DMA per k-block (1 MB each) so the first stage-1
                    # matmul group only waits for a quarter of the tile's x;
                    # xt is pre-laid-out so each partition's slice is one
                    # contiguous 8 KB descriptor
                    for k in range(NB1):
                        nc.sync.dma_start(xk[:, k, :, :], xt[bt, :, k, :, :])
                o1 = [
                    mid.tile([P, NB2, BT], bf16, tag="o1a", name="o1a"),
                    mid.tile([P, NB2, BT], bf16, tag="o1b", name="o1b"),
                ]
                if nmov or mm_only:
                    # timing-bisect modes leave parts of o1 unwritten; zero
                    # it so stage 2 never reads uninitialized SBUF
                    nc.gpsimd.memset(o1[0][:, :, :], 0.0)
                    nc.gpsimd.memset(o1[1][:, :, :], 0.0)
                # constant-1.0 rows facing w2's bias row: gpsimd needs a
                # 32-aligned start partition, so set 96:128 and let the k=3
                # copy overwrite 96:112 with real data; rows 112:126 face
                # w2 zero-padding rows and 127 faces the bias row
                nc.gpsimd.memset(o1[1][96:2 * B1P, :, :], 1.0)
                nm = nmov or BT
                for k in range(NB1):
                    pq = ps1.tile([P, 2, BT], f32, tag="pq", name="pq")
                    for qc in range(2):
                        for pc in range(PC):
                            nc.tensor.matmul(
                                pq[:, qc, 0:nm],
                                w1_sb[:, k, pc, qc * P:(qc + 1) * P],
                                xk[:, k, pc, 0:nm],
                                start=(pc == 0), stop=(pc == PC - 1),
                            )
                    half = k // 2
                    r0 = (k % 2) * B1P
                    nrow = B1P if not (half == 1 and k % 2 == 1) else B1
                    for l in ([] if mm_only else list(range(NB2))):
                        dst = o1[half][r0:r0 + nrow, l, 0:nm]
                        src = pq[(l % 2) * B1P:(l % 2) * B1P + nrow, l // 2, 0:nm]
                        if l % 2 == 0:
                            nc.vector.tensor_copy(out=dst, in_=src)
                        else:
                            nc.scalar.copy(dst, src)
                # ---- stage 2 ----
                nm2 = nmov or 512
                for bi in range(BT // P):
                    b0 = bt * BT + bi * P
                    bloc = slice(bi * P, (bi + 1) * P)
                    ob = outp.tile([P, NB2, S], bf16, tag="ob", name="ob")
                    for l in range(NB2):
                        # one 2-bank psum group per l: both s-halves
                        # accumulate both r-halves before a single
                        # evacuation, halving psum group boundaries
                        ps = ps2.tile([P, 2, 512], f32, tag="ps2", name="ps2")
                        for half in range(2):
                            for sh in range(S // 512):
                                ns2 = slice(sh * 512, sh * 512 + nm2)
                                nc.tensor.matmul(
                                    ps[:, sh, 0:nm2], o1[half][:, l, bloc],
                                    w2_sb[:, l, half, ns2],
                                    start=(half == 0), stop=(half == 1),
                                )
                        for sh in ([] if mm_only else list(range(S // 512))):
                            ns2 = slice(sh * 512, sh * 512 + nm2)
                            if (l + sh) % 2 == 0:
                                nc.vector.tensor_copy(
                                    out=ob[:, l, ns2], in_=ps[:, sh, 0:nm2])
                            else:
                                nc.scalar.copy(ob[:, l, ns2], ps[:, sh, 0:nm2])
                    if not skip_out:
                        # all output stores on the Pool (SWDGE) ring: the SP
                        # ring is FIFO, so stores there would delay the next
                        # tile's x loads behind them
                        nc.gpsimd.dma_start(out[b0:b0 + P, :], ob[:])
                return ob, o1

            if hw_loop and reps > 1:
                # hardware loop over reps: constant NEFF size, instructions
                # fetched once -- the For_i reset barrier drains the
                # pipeline between reps
                with tc.For_i(0, reps):
                    for bt in range(NBT):
                        ob, o1 = one_tile(bt)
            else:
                for it in range(NBT * reps):
                    ob, o1 = one_tile(it % NBT)
            if skip_out:
                # keep the output tensor written so the NEFF retains it
                if mm_only:
                    nc.sync.dma_start(out[0:P, 0:512], o1[0][:, 0, 0:512])
                else:
                    nw = nmov or 512
                    nc.sync.dma_start(out[0:P, 0:nw], ob[:, 0, 0:nw])


def _build(reps=1, x_resident=False, skip_out=False, nmov=None,
           hw_loop=False, mm_only=False):
    import concourse.bacc as bacc
    import concourse.mybir as mybir

    # Bacc (not plain Bass): its compile() legalizes semaphore waits
    # (move_matmul_waits_to_ldweights + generate_event_semaphores) --
    # walrus allows at most one sync wait per instruction.
    suffix = ("_xr" if x_resident else "") + ("_no" if skip_out else "")
    if nmov:
        suffix += f"_n{nmov}"
    if hw_loop:
        suffix += "_hl"
    if mm_only:
        suffix += "_mm"
    nc = bacc.Bacc(name=f"bfly_r{reps}{suffix}")
    bf16 = mybir.dt.bfloat16
    xt = nc.dram_tensor("xt", [NBT, P, NB1, PC, BT], bf16,
                        kind="ExternalInput")
    w1t = nc.dram_tensor("w1t", [NB1, IN_F // NB1, NQP], bf16, kind="ExternalInput")
    w2t = nc.dram_tensor("w2t", [NB2, NRP, S], bf16, kind="ExternalInput")
    bias = nc.dram_tensor("bias", [OUT_F], mybir.dt.float32, kind="ExternalInput")
    out = nc.dram_tensor("out", [B_LOCAL, OUT_F], bf16, kind="ExternalOutput")
    _emit(nc, xt[:], w1t[:], w2t[:], bias[:], out[:], reps=reps,
          x_resident=x_resident, skip_out=skip_out, nmov=nmov,
          hw_loop=hw_loop, mm_only=mm_only)
    nc.compile()
    return nc


def get_nc(reps=1, x_resident=False, skip_out=False, nmov=None,
           hw_loop=False, mm_only=False):
    key = ("nc", reps, x_resident, skip_out, nmov, hw_loop, mm_only)
    if key not in _CACHE:
        _CACHE[key] = _build(reps, x_resident, skip_out, nmov, hw_loop,
                             mm_only)
    return _CACHE[key]


def prep_weights(w1_bfly, w2_bfly, bias):
    """Pad the per-block width 48 -> 64, transpose for the device layout,
    cast to bf16, and plant bias in w2t's last padding row."""
    bf16 = _np_bf16()
    w1t = np.zeros((NB1, IN_F // NB1, NQP), dtype=np.float32)
    w1t_v = w1t.reshape(NB1, IN_F // NB1, NB2, B1P)
    w1t_v[:, :, :, :B1] = (
        np.asarray(w1_bfly, np.float32)
        .transpose(0, 2, 1).reshape(NB1, IN_F // NB1, NB2, B1)
    )
    w2t = np.zeros((NB2, NRP, S), dtype=np.float32)
    w2t_v = w2t.reshape(NB2, NB1, B1P, S)
    w2t_v[:, :, :B1, :] = (
        np.asarray(w2_bfly, np.float32)
        .transpose(0, 2, 1).reshape(NB2, NB1, B1, S)
    )
    w2t[:, NRP - 1, :] = np.asarray(bias, np.float32).reshape(NB2, S)
    return w1t.astype(bf16), w2t.astype(bf16)


def _prep_inputs(x, w1_bfly, w2_bfly, bias):
    bf16 = _np_bf16()
    bias = np.ascontiguousarray(np.asarray(bias, np.float32))
    w1t, w2t = prep_weights(w1_bfly, w2_bfly, bias)
    xb = np.asarray(x, np.float32).astype(bf16)
    in_maps = []
    for c in range(N_CORES):
        xc = xb[c * B_LOCAL:(c + 1) * B_LOCAL]
        # [bt, pi, k, pc, b]: each partition's per-tile slice lands as one
        # contiguous run in DRAM, so x DMAs use 8 KB descriptors
        xs = np.ascontiguousarray(
            xc.reshape(NBT, BT, NB1, PC, P).transpose(0, 4, 2, 3, 1))
        in_maps.append({"xt": xs, "w1t": w1t, "w2t": w2t, "bias": bias})
    return in_maps


def kernel(x, w1_bfly, w2_bfly, bias):
    from concourse.bass_utils import run_bass_kernel_spmd

    nc = get_nc()
    in_maps = _prep_inputs(np.asarray(x), np.asarray(w1_bfly),
                           np.asarray(w2_bfly), np.asarray(bias))
    res = run_bass_kernel_spmd(nc, in_maps, list(range(N_CORES)), trace=False)
    return np.concatenate(
        [np.asarray(res.results[c]["out"], np.float32) for c in range(N_CORES)],
        axis=0)
